# revision 2
# baseline (speedup 1.0000x reference)
"""CARAFE kernel for 8 TRN2 NeuronCores (Bass/Tile, SPMD).

Algebraic structure (see the reference):
    k0   = w_comp @ x + b_comp                  1x1 conv
    kc   = w_ker (*) k0 + b_ker                 3x3 conv -> (102400, H, W)
    k    = softmax(kc.reshape(4, 25600, H, W), axis=1)
    ksum = k.sum(axis=1)                        sum of softmax over its own
                                                axis == 1 (exactly, up to fp
                                                rounding ~1e-6)
    out  = (x[:, :, None] * ksum[:, None]).reshape(1, C, 2H, 2W)

The softmax is summed over the very axis it normalizes, so ksum == 1 and the
whole conv/softmax pipeline cancels out of the output: out is exactly x with
each channel plane replicated scale^2 = 4 times (row-major reshape, not a
pixel shuffle). The fp deviation |ksum - 1| ~ 1e-6 is orders of magnitude
below the 2e-2 gate, so the kernel computes the broadcast directly.

Device work is pure data movement. Sharding: core k owns 32 of the 256
channels; it DMA-replicates its (32, 1024) x-shard 4x into its
(32, 4, 1024) output shard (512 KB written per core, 4 MB total = the full
output). Host assembly is a reshape/concatenate only.
"""

import numpy as np

import concourse.bass as bass
import concourse.mybir as mybir
import concourse.tile as tile
from concourse import bacc
from concourse.bass_utils import run_bass_kernel_spmd

F32 = mybir.dt.float32

# Problem constants
C, H, W = 256, 32, 32
S2 = 4                    # scale^2 replication factor
NPIX = H * W              # 1024
NCORES = 8
CPC = C // NCORES         # 32 channels per core


def build():
    nc = bacc.Bacc("TRN2", target_bir_lowering=False, debug=False,
                   num_devices=NCORES)

    xin = nc.dram_tensor("xin", [CPC, NPIX], F32, kind="ExternalInput")
    out = nc.dram_tensor("out", [CPC, S2, NPIX], F32, kind="ExternalOutput")

    with tile.TileContext(nc) as tc:
        for s in range(S2):
            nc.sync.dma_start(out.ap()[:, s, :], xin.ap())

    nc.compile()
    return nc


_NC = None


def _get_nc():
    global _NC
    if _NC is None:
        _NC = build()
    return _NC


def prep_inputs(x, w_comp, b_comp, w_ker, b_ker):
    x = np.asarray(x, dtype=np.float32).reshape(C, NPIX)
    return [
        {"xin": np.ascontiguousarray(x[k * CPC:(k + 1) * CPC])}
        for k in range(NCORES)
    ]


def assemble(results):
    # results[k]["out"]: (CPC, S2, NPIX); channel plane = S2 copies of the
    # x plane back to back, which is exactly the row-major (2H, 2W) reshape.
    full = np.concatenate([results[k]["out"] for k in range(NCORES)], axis=0)
    return np.ascontiguousarray(full).reshape(1, C, 2 * H, 2 * W)


def run(in_maps, trace=False, **kw):
    nc = _get_nc()
    return run_bass_kernel_spmd(nc, in_maps, list(range(NCORES)), trace=trace, **kw)


def kernel(x, w_comp, b_comp, w_ker, b_ker):
    in_maps = prep_inputs(x, w_comp, b_comp, w_ker, b_ker)
    res = run(in_maps)
    return assemble(res.results)


# revision 3
# speedup vs baseline: 1.1545x; 1.1545x over previous
"""CARAFE kernel for 8 TRN2 NeuronCores (Bass, SPMD).

Algebraic structure (see the reference):
    k0   = w_comp @ x + b_comp                  1x1 conv
    kc   = w_ker (*) k0 + b_ker                 3x3 conv -> (102400, H, W)
    k    = softmax(kc.reshape(4, 25600, H, W), axis=1)
    ksum = k.sum(axis=1)                        == 1: sum of a softmax over
                                                its own axis (fp dev ~1e-6)
    out  = (x[:, :, None] * ksum[:, None]).reshape(1, C, 2H, 2W)

The softmax is summed over the very axis it normalizes, so ksum == 1 and the
conv/softmax pipeline cancels out of the output: out is exactly x with each
channel plane replicated scale^2 = 4 times (row-major reshape, not a pixel
shuffle). The fp deviation |ksum - 1| ~ 1e-6 sits orders of magnitude below
the 2e-2 gate, so the kernel computes the broadcast directly.

Device work is pure data movement. Sharding: core k owns 32 of the 256
channels and writes its full (32, 4, 1024) output shard (512 KB; 4 MB total
across cores = the entire output). Implementation choices, all measured on
HW (exec window = gauge first->last useful time, teardown included):

  * Raw Bass, no TileContext: the tile entry/exit barriers + drain cost
    ~1.3 us inside the measured window for a 2-instruction program.
  * Two DRAM->DRAM DMAs with a stride-0 (broadcast) source AP, one on each
    HWDGE engine (sync + scalar), each writing 2 of the 4 copies. The
    [32ch x 4KB] access pattern yields 4 KB descriptors that spray across
    all 16 SDMA engines (large contiguous descriptors would pin single
    engines at ~27 GB/s: measured 22 us vs 11.5 us). SBUF staging loses:
    the in-DMA completion receipt serializes ahead of the out-DMAs.
  * Per-engine completion semaphores so each engine reaches the end
    barrier on its own DMA's receipt.

Measured ~11.5-12.3 us vs the 282.7 us full-conv baseline; ~6.5 us of the
window is fixed walrus NEFF teardown (a serialized 254-semaphore reset,
~5.9 us of it on the PE sequencer), which bounds any kernel from below.
"""

import numpy as np

import concourse.bass as bass
import concourse.mybir as mybir
from concourse import bacc
from concourse.bass_utils import run_bass_kernel_spmd

F32 = mybir.dt.float32

# Problem constants
C, H, W = 256, 32, 32
S2 = 4                    # scale^2 replication factor
NPIX = H * W              # 1024
NCORES = 8
CPC = C // NCORES         # 32 channels per core


def build():
    nc = bacc.Bacc("TRN2", target_bir_lowering=False, debug=False,
                   num_devices=NCORES)

    xin = nc.dram_tensor("xin", [CPC, NPIX], F32, kind="ExternalInput")
    out = nc.dram_tensor("out", [CPC, S2, NPIX], F32, kind="ExternalOutput")

    sem_a = nc.alloc_semaphore("dma_done_a")
    sem_b = nc.alloc_semaphore("dma_done_b")
    src = xin.ap().unsqueeze(1).broadcast_to([CPC, S2, NPIX])
    nc.sync.dma_start(out.ap()[:, 0:2, :], src[:, 0:2, :]).then_inc(sem_a, 16)
    nc.scalar.dma_start(out.ap()[:, 2:4, :], src[:, 2:4, :]).then_inc(sem_b, 16)
    nc.sync.wait_ge(sem_a, 16)
    nc.scalar.wait_ge(sem_b, 16)

    nc.compile()
    return nc


_NC = None


def _get_nc():
    global _NC
    if _NC is None:
        _NC = build()
    return _NC


def prep_inputs(x, w_comp, b_comp, w_ker, b_ker):
    x = np.asarray(x, dtype=np.float32).reshape(C, NPIX)
    return [
        {"xin": np.ascontiguousarray(x[k * CPC:(k + 1) * CPC])}
        for k in range(NCORES)
    ]


def assemble(results):
    # results[k]["out"]: (CPC, S2, NPIX); channel plane = S2 copies of the
    # x plane back to back, which is exactly the row-major (2H, 2W) reshape.
    full = np.concatenate([results[k]["out"] for k in range(NCORES)], axis=0)
    return np.ascontiguousarray(full).reshape(1, C, 2 * H, 2 * W)


def run(in_maps, trace=False, **kw):
    nc = _get_nc()
    return run_bass_kernel_spmd(nc, in_maps, list(range(NCORES)), trace=trace, **kw)


def kernel(x, w_comp, b_comp, w_ker, b_ker):
    in_maps = prep_inputs(x, w_comp, b_comp, w_ker, b_ker)
    res = run(in_maps)
    return assemble(res.results)
